# revision 1
# baseline (speedup 1.0000x reference)
"""Self-attention scores kernel for Trainium2, 8-core SPMD.

Computes softmax((x@Wq+bq) @ (x@Wq+bq)^T / sqrt(64)) per head
(reference reuses the query projection for k, bug-for-bug).

Sharding: 32 (batch, head) pairs split 4-per-core across 8 cores.
Core c handles batch c//4, heads 4*(c%4) .. 4*(c%4)+3.
Each core gets x[b]^T (host-transposed), its Wq column slice, and its
bias slice; it computes q^T = Wq_slice^T @ x^T (+bias), then per head
the [2048, 2048] score block + row softmax, streaming 1 MiB row-blocks
back to HBM.
"""

import numpy as np

import concourse.bass as bass
import concourse.mybir as mybir
import concourse.tile as tile
from concourse import bacc
from concourse.bass_utils import run_bass_kernel_spmd

B = 2
S = 2048
D = 1024
H = 16
HS = 64
N_CORES = 8
HEADS_PER_CORE = 4  # 2 pairs of 2 heads (pair = 128 partitions)
KK = D // 128  # 8 k-tiles for the projection contraction
NQ = S // 128  # 16 q row-blocks per head
NC_ = S // 512  # 4 key chunks of 512

# Matmul input dtype: float32 is exact but 4 cycles/row on the PE;
# float32r runs at full rate for N>=256 with relaxed (tf32-like)
# precision. The BIR verifier requires fp32r matmul operands to be
# *produced* as fp32r, so the input DRAM tensors and SBUF tiles feeding
# the PE are declared float32r (numpy binding is still float32).
MM_DT = mybir.dt.float32r

# Projection inputs in fp16: 11-bit mantissa matches fp32r's effective
# precision while halving the serial input-load time at kernel start.
IN_DT = mybir.dt.float16

F32 = mybir.dt.float32


def _build():
    nc = bacc.Bacc("TRN2", target_bir_lowering=False, debug=False)
    xT = nc.dram_tensor("xT", [D, S], IN_DT, kind="ExternalInput").ap()
    WqS = nc.dram_tensor("WqS", [D, HEADS_PER_CORE * HS], IN_DT, kind="ExternalInput").ap()
    bqS = nc.dram_tensor("bqS", [128, 2], F32, kind="ExternalInput").ap()
    out = nc.dram_tensor("out", [HEADS_PER_CORE, S, S], F32, kind="ExternalOutput").ap()

    with tile.TileContext(nc) as tc:
        with (
            tc.tile_pool(name="consts", bufs=1) as consts,
            tc.tile_pool(name="qt", bufs=2) as qt_pool,
            tc.tile_pool(name="xt", bufs=KK) as xt_pool,
            tc.tile_pool(name="ps_proj", bufs=2, space="PSUM") as ps_proj,
            tc.tile_pool(name="ps_sc", bufs=3, space="PSUM") as ps_sc,
            tc.tile_pool(name="et", bufs=6) as et_pool,
            tc.tile_pool(name="small", bufs=8) as small,
        ):
            w = consts.tile([128, KK, HEADS_PER_CORE * HS], IN_DT)
            nc.sync.dma_start(out=w[:], in_=WqS.rearrange("(kk p) c -> p kk c", p=128))
            bias = consts.tile([128, 2], F32)
            nc.sync.dma_start(out=bias[:], in_=bqS)

            # x^T streamed as 8 independent k-tiles so projection matmuls
            # can start as soon as each tile lands.
            xts = []
            for kk in range(KK):
                xtt = xt_pool.tile([128, S], IN_DT, tag="xt")
                nc.sync.dma_start(out=xtt[:], in_=xT[kk * 128 : (kk + 1) * 128, :])
                xts.append(xtt)

            # ---- Projection for one head-pair ----
            def project(g):
                qtg = qt_pool.tile([128, S], MM_DT, tag="qt")
                for n in range(NC_):
                    ps = ps_proj.tile([128, 512], F32, tag="pp")
                    for kk in range(KK):
                        nc.tensor.matmul(
                            ps[:],
                            lhsT=w[:, kk, g * 128 : (g + 1) * 128],
                            rhs=xts[kk][:, n * 512 : (n + 1) * 512],
                            start=(kk == 0),
                            stop=(kk == KK - 1),
                        )
                    nc.vector.tensor_scalar_add(
                        qtg[:, n * 512 : (n + 1) * 512],
                        ps[:],
                        bias[:, g : g + 1],
                    )
                return qtg

            # ---- Scores + softmax for one head, streamed per row-block ----
            def score_head(h, qtg, i0=0, i1=NQ):
                pb = (h % 2) * 64
                for i in range(i0, i1):
                    lhsT = qtg[pb : pb + 64, i * 128 : (i + 1) * 128]
                    et = et_pool.tile([128, S], F32, tag="et")
                    sums2 = small.tile([128, 2], F32, tag="sm")
                    for half in range(2):
                        ps = ps_sc.tile([128, 1024], F32, tag="ps")
                        for j in (2 * half, 2 * half + 1):
                            nc.tensor.matmul(
                                ps[:, (j % 2) * 512 : (j % 2 + 1) * 512],
                                lhsT=lhsT,
                                rhs=qtg[pb : pb + 64, j * 512 : (j + 1) * 512],
                                start=True,
                                stop=True,
                            )
                        nc.scalar.activation(
                            out=et[:, half * 1024 : (half + 1) * 1024],
                            in_=ps[:],
                            func=mybir.ActivationFunctionType.Exp,
                            scale=1.0 / np.sqrt(float(HS)),
                            accum_out=sums2[:, half : half + 1],
                        )
                    recip = small.tile([128, 1], F32, tag="rc")
                    nc.vector.tensor_add(recip[:], sums2[:, 0:1], sums2[:, 1:2])
                    nc.vector.reciprocal(recip[:], recip[:])
                    nc.vector.tensor_scalar_mul(et[:], et[:], recip[:])
                    nc.sync.dma_start(
                        out=out[h, i * 128 : (i + 1) * 128, :], in_=et[:]
                    )

            # Emission order sets Tile's scheduling priority: get pair-0's
            # output stream going first; pair-1's projection then fills PE
            # idle slots during streaming.
            qt0 = project(0)
            score_head(0, qt0)
            score_head(1, qt0)
            qt1 = project(1)
            score_head(2, qt1)
            score_head(3, qt1)
    nc.compile()
    return nc


_NC_CACHE = None


def kernel(x, Wq, bq):
    global _NC_CACHE
    x = np.asarray(x, dtype=np.float32)
    Wq = np.asarray(Wq, dtype=np.float32)
    bq = np.asarray(bq, dtype=np.float32)
    assert x.shape == (B, S, D) and Wq.shape == (D, D) and bq.shape == (D,)

    if _NC_CACHE is None:
        _NC_CACHE = _build()
    nc = _NC_CACHE

    xTs = [np.ascontiguousarray(x[b].T.astype(np.float16)) for b in range(B)]
    Wq16 = Wq.astype(np.float16)
    in_maps = []
    for c in range(N_CORES):
        b, hg = divmod(c, N_CORES // B)
        h0 = hg * HEADS_PER_CORE
        in_maps.append(
            {
                "xT": xTs[b],
                "WqS": np.ascontiguousarray(Wq16[:, h0 * HS : (h0 + HEADS_PER_CORE) * HS]),
                "bqS": np.ascontiguousarray(
                    bq[h0 * HS : (h0 + HEADS_PER_CORE) * HS].reshape(2, 128).T
                ),
            }
        )

    res = run_bass_kernel_spmd(nc, in_maps, core_ids=list(range(N_CORES)))

    full = np.empty((B, H, S, S), dtype=np.float32)
    for c in range(N_CORES):
        b, hg = divmod(c, N_CORES // B)
        h0 = hg * HEADS_PER_CORE
        full[b, h0 : h0 + HEADS_PER_CORE] = res.results[c]["out"]
    return full



# revision 5
# speedup vs baseline: 1.2550x; 1.2550x over previous
"""Self-attention scores kernel for Trainium2, 8-core SPMD.

Computes softmax((x@Wq+bq) @ (x@Wq+bq)^T / sqrt(64)) per head
(reference reuses the query projection for k, bug-for-bug).

Sharding: 32 (batch, head) pairs split 4-per-core across 8 cores.
Core c handles batch c//4, heads 4*(c%4) .. 4*(c%4)+3.

v2 pipeline per 128-row block:
  PE:  scores into one [128, 2048] f32 PSUM tile (2 matmuls, N=1024)
  ACT: exp(z/8 - 9) PSUM -> SBUF fp16 (single call; row sums via accum)
  DVE: reciprocal of sums; normalize in-place fp16 @4x
  DMA: two blocks batched per 1 MiB fp16 store
Host converts the fp16 result back to f32 (gather/unshard step).
"""

import numpy as np

import concourse.bass as bass
import concourse.mybir as mybir
import concourse.tile as tile
from concourse import bacc
from concourse.bass_utils import run_bass_kernel_spmd

B = 2
S = 2048
D = 1024
H = 16
HS = 64
N_CORES = 8
HEADS_PER_CORE = 4  # 2 pairs of 2 heads (pair = 128 partitions)
KK = D // 128  # 8 k-tiles for the projection contraction
NQ = S // 128  # 16 q row-blocks per head

IN_DT = mybir.dt.float16
F16 = mybir.dt.float16
F32 = mybir.dt.float32

# exp(z - SHIFT) keeps all values in fp16 range: z = q.k/8 <= max|q|^2/8,
# which concentrates near 8 and exceeds SHIFT + 11 (fp16 overflow) with
# probability ~1e-8. Softmax normalization cancels the shift exactly.
SHIFT = 9.0


def _build():
    nc = bacc.Bacc("TRN2", target_bir_lowering=False, debug=False)
    xT = nc.dram_tensor("xT", [D, S], IN_DT, kind="ExternalInput").ap()
    WqS = nc.dram_tensor("WqS", [D, HEADS_PER_CORE * HS], IN_DT, kind="ExternalInput").ap()
    bqS = nc.dram_tensor("bqS", [128, 2], F32, kind="ExternalInput").ap()
    out = nc.dram_tensor("out", [HEADS_PER_CORE, S, S], F16, kind="ExternalOutput").ap()

    with tile.TileContext(nc) as tc:
        with (
            tc.tile_pool(name="consts", bufs=1) as consts,
            tc.tile_pool(name="qt", bufs=2) as qt_pool,
            tc.tile_pool(name="xt", bufs=KK) as xt_pool,
            tc.tile_pool(name="ps", bufs=2, space="PSUM") as ps_pool,
            tc.tile_pool(name="et", bufs=4) as et_pool,
            tc.tile_pool(name="small", bufs=8) as small,
        ):
            w = consts.tile([128, KK, HEADS_PER_CORE * HS], IN_DT)
            nc.sync.dma_start(out=w[:], in_=WqS.rearrange("(kk p) c -> p kk c", p=128))
            bias = consts.tile([128, 2], F32)
            nc.sync.dma_start(out=bias[:], in_=bqS)
            shift = consts.tile([128, 1], F32)
            nc.gpsimd.memset(shift[:], -SHIFT)

            # x^T streamed as 8 independent k-tiles so projection matmuls
            # can start as soon as each tile lands.
            xts = []
            for kk in range(KK):
                xtt = xt_pool.tile([128, S], IN_DT, tag="xt")
                nc.sync.dma_start(out=xtt[:], in_=xT[kk * 128 : (kk + 1) * 128, :])
                xts.append(xtt)

            # ---- Projection for one head-pair (fp16 q^T in SBUF) ----
            def project(g):
                qtg = qt_pool.tile([128, S], F16, tag="qt")
                for n in range(4):
                    ps = ps_pool.tile([128, S], F32, tag="ps")
                    for kk in range(KK):
                        nc.tensor.matmul(
                            ps[:, n * 512 : (n + 1) * 512],
                            lhsT=w[:, kk, g * 128 : (g + 1) * 128],
                            rhs=xts[kk][:, n * 512 : (n + 1) * 512],
                            start=(kk == 0),
                            stop=(kk == KK - 1),
                        )
                    nc.vector.tensor_scalar_add(
                        qtg[:, n * 512 : (n + 1) * 512],
                        ps[:, n * 512 : (n + 1) * 512],
                        bias[:, g : g + 1],
                    )
                return qtg

            # ---- Scores + softmax for one head, 2 row-blocks per DMA ----
            def score_head(h, qtg):
                pb = (h % 2) * 64
                for ip in range(NQ // 2):
                    et = et_pool.tile([128, 2, S], F16, tag="et")
                    for r in range(2):
                        i = 2 * ip + r
                        lhsT = qtg[pb : pb + 64, i * 128 : (i + 1) * 128]
                        ps = ps_pool.tile([128, S], F32, tag="ps")
                        for j in range(4):
                            nc.tensor.matmul(
                                ps[:, j * 512 : (j + 1) * 512],
                                lhsT=lhsT,
                                rhs=qtg[pb : pb + 64, j * 512 : (j + 1) * 512],
                                start=True,
                                stop=True,
                            )
                        sums = small.tile([128, 1], F32, tag="sm")
                        nc.scalar.activation(
                            out=et[:, r, :],
                            in_=ps[:],
                            func=mybir.ActivationFunctionType.Exp,
                            scale=1.0 / np.sqrt(float(HS)),
                            bias=shift[:],
                            accum_out=sums[:],
                        )
                        recip = small.tile([128, 1], F32, tag="rc")
                        nc.vector.reciprocal(recip[:], sums[:])
                        nc.vector.tensor_scalar_mul(et[:, r, :], et[:, r, :], recip[:])
                    nc.sync.dma_start(
                        out=out[h, ip * 256 : (ip + 1) * 256, :].rearrange(
                            "(r p) c -> p r c", p=128
                        ),
                        in_=et[:],
                    )

            # Emission order sets Tile's scheduling priority: get pair-0's
            # output stream going first; pair-1's projection then fills PE
            # idle slots during streaming.
            qt0 = project(0)
            score_head(0, qt0)
            score_head(1, qt0)
            qt1 = project(1)
            score_head(2, qt1)
            score_head(3, qt1)
    nc.compile()
    return nc


_NC_CACHE = None


def kernel(x, Wq, bq):
    global _NC_CACHE
    x = np.asarray(x, dtype=np.float32)
    Wq = np.asarray(Wq, dtype=np.float32)
    bq = np.asarray(bq, dtype=np.float32)
    assert x.shape == (B, S, D) and Wq.shape == (D, D) and bq.shape == (D,)

    if _NC_CACHE is None:
        _NC_CACHE = _build()
    nc = _NC_CACHE

    xTs = [np.ascontiguousarray(x[b].T.astype(np.float16)) for b in range(B)]
    Wq16 = Wq.astype(np.float16)
    in_maps = []
    for c in range(N_CORES):
        b, hg = divmod(c, N_CORES // B)
        h0 = hg * HEADS_PER_CORE
        in_maps.append(
            {
                "xT": xTs[b],
                "WqS": np.ascontiguousarray(Wq16[:, h0 * HS : (h0 + HEADS_PER_CORE) * HS]),
                "bqS": np.ascontiguousarray(
                    bq[h0 * HS : (h0 + HEADS_PER_CORE) * HS].reshape(2, 128).T
                ),
            }
        )

    res = run_bass_kernel_spmd(nc, in_maps, core_ids=list(range(N_CORES)))

    full = np.empty((B, H, S, S), dtype=np.float32)
    for c in range(N_CORES):
        b, hg = divmod(c, N_CORES // B)
        h0 = hg * HEADS_PER_CORE
        full[b, h0 : h0 + HEADS_PER_CORE] = res.results[c]["out"]
    return full
